# revision 1
# baseline (speedup 1.0000x reference)
"""DilatedCNN forward on 8 TRN2 NeuronCores.

Strategy: data-parallel over the sequence dim N with halo. Each core owns
M=1024 rows plus an 8-row halo on each side (8 = sum of dilations
[1,2,4,1]); with the halo, all four layers are computed fully locally —
no collectives. The activation state lives in SBUF *transposed*
(feature-major: [128 partitions = feature chunk, rows in the free dim]) so
that
  * the concat [X, X_left, X_right] is just three column-shifted views of
    the same buffer (shifts along the free dim are free),
  * the 3072-feature contraction has features on partitions as the
    TensorEngine requires for both operands,
  * each layer's output is again feature-major — ready to be the next
    layer's input with no data movement,
  * the per-feature bias is a per-partition scalar for the activation op.
Matmuls run in float32r (TF32-path, full PE rate at free-dim >= 256); the
residual state stays fp32, with a rounded fp32r copy made per layer for
the GEMM inputs. Out-of-range rows are refreshed with the `oob` vector
between layers via copy_predicated driven by per-core mask/fill inputs,
so all 8 cores run one identical program.
"""

import numpy as np

import concourse.bacc as bacc
import concourse.mybir as mybir
import concourse.tile as tile
from concourse.bass_utils import run_bass_kernel_spmd

N, DIM, NL = 8192, 1024, 4
NCORES = 8
M = N // NCORES           # rows per core
H = 8                     # halo rows each side (sum of dilations)
PAD = 4                   # zero cols so shifted reads stay in-bounds
B = M + 2 * H             # 1040 buffer rows
FB = PAD + B + PAD        # 1048 free-dim cols of the state buffer
DIL = [1, 2, 4, 1]
KT = 3 * DIM // 128       # 24 contraction tiles
DT = DIM // 128           # 8 feature tiles
# Per-layer compute windows (rows [start, start+size) of the B-row buffer),
# shrinking by the dilation each layer; all sizes even (fp32r streams
# column pairs) and >= 256 (fp32r full-rate threshold).
ROW_BLOCKS_L = [
    [(1, 346), (347, 346), (693, 346)],   # layer 1: rows [1, 1039)
    [(3, 346), (349, 344), (693, 344)],   # layer 2: rows [3, 1037)
    [(7, 342), (349, 342), (691, 342)],   # layer 3: rows [7, 1033)
    [(8, 512), (520, 512)],               # layer 4: rows [8, 1032)
]
F32 = mybir.dt.float32
F32R = mybir.dt.float32r

_CACHE = {}
LAST_RESULTS = None  # test harness reads exec_time_ns from here


def _build():
    nc = bacc.Bacc("TRN2", target_bir_lowering=False, debug=False)

    xs_d = nc.dram_tensor("XST", [128, DT, B], F32, kind="ExternalInput")
    w_d = nc.dram_tensor("WT", [NL, 2, KT, 128, 512], F32, kind="ExternalInput")
    b_d = nc.dram_tensor("BS", [128, NL * DT], F32, kind="ExternalInput")
    ml_d = nc.dram_tensor("ML", [128, DT, H], mybir.dt.uint8, kind="ExternalInput")
    fl_d = nc.dram_tensor("FL", [128, DT, H], F32, kind="ExternalInput")
    mr_d = nc.dram_tensor("MR", [128, DT, H], mybir.dt.uint8, kind="ExternalInput")
    fr_d = nc.dram_tensor("FR", [128, DT, H], F32, kind="ExternalInput")
    y_d = nc.dram_tensor("YT", [128, DT, M], F32, kind="ExternalOutput")

    with tile.TileContext(nc) as tc:
        with (
            tc.tile_pool(name="state", bufs=1) as state_pool,
            tc.tile_pool(name="wpool", bufs=1) as w_pool,
            tc.tile_pool(name="const", bufs=1) as const_pool,
            tc.tile_pool(name="tmp", bufs=4) as tmp_pool,
            tc.tile_pool(name="gps", bufs=6, space="PSUM") as gps_pool,
        ):
            S = state_pool.tile([128, DT, FB], F32)    # fp32 residual state
            R = state_pool.tile([128, DT, FB], F32R)   # rounded GEMM input

            # zero the PAD columns once; epilogues never touch them
            nc.gpsimd.memset(S[:, :, 0:PAD], 0.0)
            nc.gpsimd.memset(S[:, :, PAD + B:FB], 0.0)


            # ---- entry: host pre-transposed X -> straight DMA into S ----
            # dt0 rides the fast sync queue (it gates the first matmul);
            # the rest go via GpSimd so the weight DMAs own sync
            for dt in range(DT):
                eng = nc.sync if dt == 0 else nc.gpsimd
                eng.dma_start(S[:, dt, PAD:PAD + B], xs_d[:, dt, :])

            bs_t = const_pool.tile([128, NL * DT], F32)
            mask_l = const_pool.tile([128, DT, H], mybir.dt.uint8)
            fill_l = const_pool.tile([128, DT, H], F32)
            mask_r = const_pool.tile([128, DT, H], mybir.dt.uint8)
            fill_r = const_pool.tile([128, DT, H], F32)
            nc.gpsimd.dma_start(bs_t[:], b_d[:])
            nc.gpsimd.dma_start(mask_l[:], ml_d[:])
            nc.gpsimd.dma_start(fill_l[:], fl_d[:])
            nc.gpsimd.dma_start(mask_r[:], mr_d[:])
            nc.gpsimd.dma_start(fill_r[:], fr_d[:])

            # ---- layers ----
            w_tiles = {}
            for l, d in enumerate(DIL):
                # rounded copy of the state for this layer's GEMMs
                for dt in range(DT):
                    nc.vector.tensor_copy(R[:, dt, :], S[:, dt, :])

                row_blocks = ROW_BLOCKS_L[l]
                for h in range(2):
                    for kt in range(KT):
                        wt = w_pool.tile([128, 512], F32R, tag=f"w{h}_{kt}")
                        w_tiles[(h, kt)] = wt
                        nc.sync.dma_start(
                            wt[:], w_d[l, h, kt].bitcast(F32R)
                        )
                    def mm_g(ps, c0, nb, mtl, kt):
                        dt = kt % DT
                        grp = kt // DT
                        sh = 0 if grp == 0 else (-d if grp == 1 else d)
                        nc.tensor.matmul(
                            ps[:, 0:nb],
                            w_tiles[(h, kt)][:, mtl * 128:(mtl + 1) * 128],
                            R[:, dt, PAD + c0 + sh:PAD + c0 + sh + nb],
                            start=(kt == 0),
                            stop=(kt == KT - 1),
                        )

                    def epilogue_g(ps, c0, nb, mtl):
                        mt = h * 4 + mtl
                        tmp = tmp_pool.tile([128, 512], F32, tag="tmp",
                                            name=f"tmp{l}_{h}_{c0}_{mtl}")
                        nc.scalar.activation(
                            tmp[:, 0:nb],
                            ps[:, 0:nb],
                            mybir.ActivationFunctionType.Relu,
                            bias=bs_t[:, l * DT + mt:l * DT + mt + 1],
                            scale=0.5,
                        )
                        # S = 0.5*S + relu(0.5*cat@W + 0.5*b), in place
                        nc.vector.scalar_tensor_tensor(
                            S[:, mt, PAD + c0:PAD + c0 + nb],
                            S[:, mt, PAD + c0:PAD + c0 + nb],
                            0.5,
                            tmp[:, 0:nb],
                            mybir.AluOpType.mult,
                            mybir.AluOpType.add,
                        )

                    groups = [(c0, nb, mtl)
                              for (c0, nb) in row_blocks
                              for mtl in range(4)]
                    # The very first half-layer races the weight DMAs (one
                    # k-tile lands per ~610ns but a single accumulation group
                    # consumes one per ~145ns). Interleave the first 6 groups
                    # per k-tile: consumption 6 x ~145ns/kt stays behind the
                    # supply; 6 PSUM banks (not all 8) keeps recycling slack
                    # supply, so the PE never waits on weights.
                    n_inter = 6 if (l == 0 and h == 0) else 0
                    head = groups[:n_inter]
                    if head:
                        pss = [
                            gps_pool.tile([128, 512], F32, tag="gps",
                                          name=f"psi{i}")
                            for i in range(len(head))
                        ]
                        for kt in range(KT):
                            for i, (c0, nb, mtl) in enumerate(head):
                                mm_g(pss[i], c0, nb, mtl, kt)
                        for i, (c0, nb, mtl) in enumerate(head):
                            epilogue_g(pss[i], c0, nb, mtl)
                    for j, (c0, nb, mtl) in enumerate(groups[n_inter:]):
                        ps = gps_pool.tile([128, 512], F32, tag="gps",
                                           name=f"ps{l}_{h}_{j}")
                        for kt in range(KT):
                            mm_g(ps, c0, nb, mtl, kt)
                        epilogue_g(ps, c0, nb, mtl)

                # refresh out-of-range halo rows with oob (data-driven; only
                # the edge cores have nonzero masks)
                if l < NL - 1:
                    for dt in range(DT):
                        nc.vector.copy_predicated(
                            S[:, dt, PAD:PAD + H],
                            mask_l[:, dt, :], fill_l[:, dt, :],
                        )
                        nc.vector.copy_predicated(
                            S[:, dt, PAD + B - H:PAD + B],
                            mask_r[:, dt, :], fill_r[:, dt, :],
                        )

            # ---- exit: dump the feature-major state; host untransposes ----
            # (two DMAs per dt so each half fires as soon as its last
            # epilogue lands, instead of waiting for the whole row range)
            for dt in range(DT):
                for c0h in (0, 512):
                    nc.sync.dma_start(
                        y_d[:, dt, c0h:c0h + 512],
                        S[:, dt, PAD + H + c0h:PAD + H + c0h + 512],
                    )

    nc.compile()
    return nc


def _get_nc():
    if "nc" not in _CACHE:
        _CACHE["nc"] = _build()
    return _CACHE["nc"]


def kernel(X, Ws, bs, oob):
    global LAST_RESULTS
    X = np.ascontiguousarray(np.asarray(X, np.float32))
    Ws = np.ascontiguousarray(np.asarray(Ws, np.float32))
    bs = np.ascontiguousarray(np.asarray(bs, np.float32))
    oob = np.ascontiguousarray(np.asarray(oob, np.float32))

    nc = _get_nc()

    # host-side input prep (pure layout rearrangement)
    WT = np.ascontiguousarray(
        Ws.reshape(NL, KT, 128, 2, 512).transpose(0, 3, 1, 2, 4)
    )
    BS = np.ascontiguousarray(
        (0.5 * bs).reshape(NL, DT, 128).transpose(2, 0, 1).reshape(128, NL * DT)
    )
    oobT = np.ascontiguousarray(oob.reshape(DT, 128).T)  # [128, DT]
    fill_edge = np.repeat(oobT[:, :, None], H, axis=2)   # [128, DT, H]
    ones = np.ones((128, DT, H), np.uint8)
    zeros_m = np.zeros((128, DT, H), np.uint8)
    zeros = np.zeros((128, DT, H), np.float32)

    in_maps = []
    for c in range(NCORES):
        lo, hi = c * M - H, c * M + M + H
        xs = np.empty((B, DIM), np.float32)
        slo, shi = max(lo, 0), min(hi, N)
        xs[slo - lo:shi - lo] = X[slo:shi]
        if lo < 0:
            xs[0:-lo] = oob
        if hi > N:
            xs[B - (hi - N):] = oob
        xst = np.ascontiguousarray(
            xs.reshape(B, DT, 128).transpose(2, 1, 0))
        left_edge = c == 0
        right_edge = c == NCORES - 1
        in_maps.append({
            "XST": xst,
            "WT": WT,
            "BS": BS,
            "ML": ones if left_edge else zeros_m,
            "FL": fill_edge if left_edge else zeros,
            "MR": ones if right_edge else zeros_m,
            "FR": fill_edge if right_edge else zeros,
        })

    res = run_bass_kernel_spmd(nc, in_maps, list(range(NCORES)))
    LAST_RESULTS = res
    out = np.concatenate(
        [res.results[c]["YT"].transpose(2, 1, 0).reshape(M, DIM)
         for c in range(NCORES)],
        axis=0,
    )
    return out[None, :, :].astype(np.float32)



# revision 7
# speedup vs baseline: 1.2110x; 1.2110x over previous
"""DilatedCNN forward on 8 TRN2 NeuronCores — Winograd F(2,3) version.

Strategy: data-parallel over the sequence dim N with an 8-row halo per
side (8 = sum of dilations [1,2,4,1]); all four layers run fully locally,
no collectives. The activation state lives in SBUF feature-major
([128 partitions = feature chunk, rows in the free dim]) as in the
baseline.

Each layer is a 3-tap dilated conv over rows:
    y[n] = X[n-d]@W1 + X[n]@W0 + X[n+d]@W2
computed with Winograd F(2,3), which shares products between each output
pair (n, n+d) and cuts tensor-engine work from 3 to 2 GEMM-rows per
output row (1.5x FLOP reduction):
    z0 = X[n-d] - X[n+d]   m0 = z0 @ W1
    z1 = X[n]   + X[n+d]   m1 = z1 @ (W1+W0+W2)/2
    z2 = X[n+d] - X[n]     m2 = z2 @ (W1-W0+W2)/2
    z3 = X[n]   - X[n+2d]  m3 = z3 @ W2
    y[n]   = m0 + m1 + m2
    y[n+d] = m1 - m2 - m3
The z tensors are built by the vector engine with strided row reads
(free-dim shifts are cheap in the feature-major layout) and stored bf16;
the transformed weights are bf16, so the GEMMs run at full PE rate with
fp32 PSUM accumulation. The per-layer compute windows are padded to a
whole number of pairs per dilation class; the extra edge rows compute
garbage that is provably never read downstream. Out-of-range rows are
refreshed with `oob` between layers via copy_predicated, so all 8 cores
run one identical program.
"""

import numpy as np
import ml_dtypes

import concourse.bacc as bacc
import concourse.mybir as mybir
import concourse.tile as tile
from concourse.bass_utils import run_bass_kernel_spmd

N, DIM, NL = 8192, 1024, 4
NCORES = 8
M = N // NCORES           # rows per core
H = 8                     # halo rows each side (sum of dilations)
B = M + 2 * H             # 1040 buffer rows
DT = DIM // 128           # 8 feature tiles
DIL = [1, 2, 4, 1]
# Per-layer compute windows [lo, hi) over the B-row buffer. Rows outside
# the strictly-needed window (baseline: [1,1039),[3,1037),[7,1033),[8,1032))
# are computed with stale inputs but never read downstream; windows are
# chosen so (hi-lo) splits into an integer number of pairs per class.
WIN = [(1, 1039), (2, 1038), (4, 1036), (8, 1032)]
ZMAX = 520                # max pairs per layer (layer 1: 519)
F32 = mybir.dt.float32
BF16 = mybir.dt.bfloat16

# Per-layer block split of the pair index space (class-major order).
# block = (z_off, nb, n_classes, cnt_per_class, base_row)
# rows(y0) = base + c*1 + j*2d  (c < n_classes, j < cnt)
def _blocks(l):
    d = DIL[l]
    lo, hi = WIN[l]
    P = (hi - lo) // (2 * d)          # pairs per class
    if d == 1:
        half = (P + 1) // 2
        return [(0, half, 1, half, lo),
                (half, P - half, 1, P - half, lo + 2 * half)]
    if d == 2:
        return [(0, P, 1, P, lo), (P, P, 1, P, lo + 1)]
    # d == 4: two classes per block
    return [(0, 2 * P, 2, P, lo), (2 * P, 2 * P, 2, P, lo + 2)]


BLOCKS = [_blocks(l) for l in range(NL)]

_CACHE = {}
LAST_RESULTS = None  # test harness reads exec_time_ns from here


def _src_ap(S, dt, off, d, P):
    """[128, d, P] view of S rows off + c + j*2d (c<d classes, j<P)."""
    if d == 1:
        return S[:, dt, off:off + 2 * P:2]
    a = S[:, dt, off:off + 2 * d * P].rearrange("p (j e) -> p j e", j=P)
    return a[:, :, 0:d].transpose([0, 2, 1])


def _wb_ap(S, mt, base, ncls, cnt, d):
    """[128, ncls, cnt] view of S rows base + c + j*2d."""
    if ncls == 1:
        return S[:, mt, base:base + 2 * d * cnt:2 * d]
    a = S[:, mt, base:base + 2 * d * cnt].rearrange("p (j e) -> p j e", j=cnt)
    return a[:, :, 0:ncls].transpose([0, 2, 1])


def _build():
    nc = bacc.Bacc("TRN2", target_bir_lowering=False, debug=False)

    xs_d = nc.dram_tensor("XST", [128, DT, B], F32, kind="ExternalInput")
    w_d = nc.dram_tensor("WGH", [NL, 2, 128, 4, DT, 512], BF16,
                         kind="ExternalInput")
    b_d = nc.dram_tensor("BS", [128, NL * DT], F32, kind="ExternalInput")
    ml_d = nc.dram_tensor("ML", [128, DT, H], mybir.dt.uint8, kind="ExternalInput")
    fl_d = nc.dram_tensor("FL", [128, DT, H], F32, kind="ExternalInput")
    mr_d = nc.dram_tensor("MR", [128, DT, H], mybir.dt.uint8, kind="ExternalInput")
    fr_d = nc.dram_tensor("FR", [128, DT, H], F32, kind="ExternalInput")
    y_d = nc.dram_tensor("YT", [128, DT, M], F32, kind="ExternalOutput")

    Relu = mybir.ActivationFunctionType.Relu
    ADD = mybir.AluOpType.add
    SUB = mybir.AluOpType.subtract
    MUL = mybir.AluOpType.mult

    with tile.TileContext(nc) as tc:
        with (
            tc.tile_pool(name="state", bufs=1) as state_pool,
            tc.tile_pool(name="wpool", bufs=2) as w_pool,
            tc.tile_pool(name="const", bufs=1) as const_pool,
            tc.tile_pool(name="tmp", bufs=6) as tmp_pool,
            tc.tile_pool(name="gps", bufs=2, space="PSUM") as gps_pool,
        ):
            # +8 slack cols: strided-view construction slices up to
            # off + 2*d*P before narrowing; the pattern never reads them
            S = state_pool.tile([128, DT, B + 8], F32, name="S")
            # double-buffered z sets (ping-pong across layers) so layer
            # l+1's z-build never WARs layer l's matmul reads
            Z = [[state_pool.tile([128, DT, ZMAX], BF16, name=f"z{pp}_{c}")
                  for c in range(4)]
                 for pp in range(2)]

            # ---- entry: host pre-transposed X -> straight DMA into S ----
            for dt in range(DT):
                eng = nc.sync if dt == 0 else nc.gpsimd
                eng.dma_start(S[:, dt, 0:B], xs_d[:, dt, :])

            bs_t = const_pool.tile([128, NL * DT], F32)
            mask_l = const_pool.tile([128, DT, H], mybir.dt.uint8)
            fill_l = const_pool.tile([128, DT, H], F32)
            mask_r = const_pool.tile([128, DT, H], mybir.dt.uint8)
            fill_r = const_pool.tile([128, DT, H], F32)
            nc.gpsimd.dma_start(bs_t[:], b_d[:])
            nc.gpsimd.dma_start(mask_l[:], ml_d[:])
            nc.gpsimd.dma_start(fill_l[:], fl_d[:])
            nc.gpsimd.dma_start(mask_r[:], mr_d[:])
            nc.gpsimd.dma_start(fill_r[:], fr_d[:])

            # ---- layers ----
            for l, d in enumerate(DIL):
                lo, hi = WIN[l]
                P = (hi - lo) // (2 * d)
                Zl = (hi - lo) // 2
                zs = Z[l % 2]

                # z build: dt ascending so dt7 (which waits on the last
                # epilogues of the previous layer) is produced last, and
                # the PE's kt-ascending consumption overlaps the tail of
                # the previous layer.
                zdefs = [(lo - d, lo + d, SUB), (lo, lo + d, ADD),
                         (lo + d, lo, SUB), (lo, lo + 2 * d, SUB)]
                for dt in range(DT):
                    for comp, (o0, o1, op) in enumerate(zdefs):
                        out = zs[comp][:, dt, 0:Zl]
                        if d > 1:
                            out = out.rearrange("p (c j) -> p c j", c=d)
                        nc.vector.tensor_tensor(
                            out,
                            _src_ap(S, dt, o0, d, P),
                            _src_ap(S, dt, o1, d, P),
                            op,
                        )

                for h in range(2):
                    wq = w_pool.tile([128, 4, DT, 512], BF16, tag="wq",
                                     name=f"wq{l}_{h}")
                    for kt in range(DT):
                        for comp in range(4):
                            nc.sync.dma_start(
                                wq[:, comp, kt, :],
                                w_d[l, h][:, comp, kt, :],
                            )
                    for mtl in range(4):
                        mt = 4 * h + mtl
                        for bi, (z_off, nb, ncls, cnt, base) in enumerate(BLOCKS[l]):
                            ps = gps_pool.tile([128, 4, 512], F32, tag="gps",
                                               name=f"ps{l}_{h}_{mtl}_{bi}")
                            for kt in range(DT):
                                for comp in range(4):
                                    nc.tensor.matmul(
                                        ps[:, comp, 0:nb],
                                        wq[:, comp, kt,
                                           mtl * 128:(mtl + 1) * 128],
                                        zs[comp][:, kt, z_off:z_off + nb],
                                        start=(kt == 0),
                                        stop=(kt == DT - 1),
                                    )
                            bias = bs_t[:, l * DT + mt:l * DT + mt + 1]
                            # DVE may read at most ONE PSUM operand per op
                            # (NCC_IBVF027): ACT evacuates m1, then
                            #   T2 = m1 + m2, y0 = T2 + m0,
                            #   y1 = (2*m1 - T2) - m3
                            T1 = tmp_pool.tile([128, 512], F32, tag="tmp",
                                               name=f"t1_{l}_{h}_{mtl}_{bi}")
                            T2 = tmp_pool.tile([128, 512], F32, tag="tmp",
                                               name=f"t2_{l}_{h}_{mtl}_{bi}")
                            T3 = tmp_pool.tile([128, 512], F32, tag="tmp",
                                               name=f"t3_{l}_{h}_{mtl}_{bi}")
                            nc.scalar.activation(
                                T1[:, 0:nb], ps[:, 1, 0:nb],
                                mybir.ActivationFunctionType.Copy)
                            nc.vector.tensor_tensor(
                                T2[:, 0:nb], T1[:, 0:nb], ps[:, 2, 0:nb], ADD)
                            nc.vector.tensor_tensor(
                                T3[:, 0:nb], T2[:, 0:nb], ps[:, 0, 0:nb], ADD)
                            nc.scalar.activation(
                                T3[:, 0:nb], T3[:, 0:nb], Relu,
                                bias=bias, scale=0.5)
                            wb0 = _wb_ap(S, mt, base, ncls, cnt, d)
                            a_v = T3[:, 0:nb]
                            if ncls > 1:
                                a_v = a_v.rearrange("p (c j) -> p c j", c=ncls)
                            nc.vector.scalar_tensor_tensor(
                                wb0, wb0, 0.5, a_v, MUL, ADD)
                            nc.vector.scalar_tensor_tensor(
                                T1[:, 0:nb], T1[:, 0:nb], 2.0, T2[:, 0:nb],
                                MUL, SUB)
                            nc.vector.tensor_tensor(
                                T1[:, 0:nb], T1[:, 0:nb], ps[:, 3, 0:nb], SUB)
                            nc.scalar.activation(
                                T1[:, 0:nb], T1[:, 0:nb], Relu,
                                bias=bias, scale=0.5)
                            wb1 = _wb_ap(S, mt, base + d, ncls, cnt, d)
                            c_v = T1[:, 0:nb]
                            if ncls > 1:
                                c_v = c_v.rearrange("p (c j) -> p c j", c=ncls)
                            nc.vector.scalar_tensor_tensor(
                                wb1, wb1, 0.5, c_v, MUL, ADD)

                # refresh out-of-range halo rows with oob (data-driven; only
                # the edge cores have nonzero masks)
                if l < NL - 1:
                    for dt in range(DT):
                        nc.vector.copy_predicated(
                            S[:, dt, 0:H], mask_l[:, dt, :], fill_l[:, dt, :])
                        nc.vector.copy_predicated(
                            S[:, dt, B - H:B], mask_r[:, dt, :], fill_r[:, dt, :])

            # ---- exit: dump the feature-major state; host untransposes ----
            for dt in range(DT):
                for c0h in (0, 512):
                    nc.sync.dma_start(
                        y_d[:, dt, c0h:c0h + 512],
                        S[:, dt, H + c0h:H + c0h + 512],
                    )

    nc.compile()
    return nc


def _get_nc():
    if "nc" not in _CACHE:
        _CACHE["nc"] = _build()
    return _CACHE["nc"]


def kernel(X, Ws, bs, oob):
    global LAST_RESULTS
    X = np.ascontiguousarray(np.asarray(X, np.float32))
    Ws = np.ascontiguousarray(np.asarray(Ws, np.float32))
    bs = np.ascontiguousarray(np.asarray(bs, np.float32))
    oob = np.ascontiguousarray(np.asarray(oob, np.float32))

    nc = _get_nc()

    # Winograd weight components (taps g = (W1, W0, W2))
    W0 = Ws[:, :DIM]          # [NL, DIM, DIM]
    W1 = Ws[:, DIM:2 * DIM]
    W2 = Ws[:, 2 * DIM:]
    G = np.stack([W1, 0.5 * (W1 + W0 + W2), 0.5 * (W1 - W0 + W2), W2], axis=1)
    # [NL, 4, DIM(in), DIM(out)] -> [NL, 2, 128, 4, 8, 512]
    WGH = np.ascontiguousarray(
        G.reshape(NL, 4, DT, 128, 2, 512).transpose(0, 4, 3, 1, 2, 5)
    ).astype(ml_dtypes.bfloat16)
    BS = np.ascontiguousarray(
        (0.5 * bs).reshape(NL, DT, 128).transpose(2, 0, 1).reshape(128, NL * DT)
    )
    oobT = np.ascontiguousarray(oob.reshape(DT, 128).T)  # [128, DT]
    fill_edge = np.repeat(oobT[:, :, None], H, axis=2)   # [128, DT, H]
    ones = np.ones((128, DT, H), np.uint8)
    zeros_m = np.zeros((128, DT, H), np.uint8)
    zeros = np.zeros((128, DT, H), np.float32)

    in_maps = []
    for c in range(NCORES):
        lo, hi = c * M - H, c * M + M + H
        xs = np.empty((B, DIM), np.float32)
        slo, shi = max(lo, 0), min(hi, N)
        xs[slo - lo:shi - lo] = X[slo:shi]
        if lo < 0:
            xs[0:-lo] = oob
        if hi > N:
            xs[B - (hi - N):] = oob
        xst = np.ascontiguousarray(
            xs.reshape(B, DT, 128).transpose(2, 1, 0))
        left_edge = c == 0
        right_edge = c == NCORES - 1
        in_maps.append({
            "XST": xst,
            "WGH": WGH,
            "BS": BS,
            "ML": ones if left_edge else zeros_m,
            "FL": fill_edge if left_edge else zeros,
            "MR": ones if right_edge else zeros_m,
            "FR": fill_edge if right_edge else zeros,
        })

    res = run_bass_kernel_spmd(nc, in_maps, list(range(NCORES)))
    LAST_RESULTS = res
    out = np.concatenate(
        [res.results[c]["YT"].transpose(2, 1, 0).reshape(M, DIM)
         for c in range(NCORES)],
        axis=0,
    )
    return out[None, :, :].astype(np.float32)
